# revision 46
# baseline (speedup 1.0000x reference)
"""AttentionSequencePoolingLayer Trainium2 kernel (8-core data parallel).

B=2048, S=200, D=64, H1=64, H2=16. Batch sharded 256/core.

Rows are sorted by seq_length on the host and packed into 16 groups per core
with static token capacities CAPS (multiples of 16).  Tokens beyond a group's
capacity are provably masked (sigmoid(-inf)=0) so skipping them is exact.

Per group of 16 rows (8 row-pairs, 4 "units" of 2 pairs each):
  gf/gp: token-major SWDGE cast-loads (f32->bf16), XBAR transpose -> kt
         [128=(bhat,d), pair, tok].
  unit u (pairs 2u,2u+1), w = 2c tokens:
    x1 = qW (preload via K=2 matmul) + blk(Wk)^T kt + blk(Wqk)^T (q*kt)
    p1 = sigmoid(s1*x1 + b1)        [ACT, uniform per-partition scale/bias]
    g  = a1 + (1-a1)*p1             [DVE TS]
    h1 = x1 * g                     [DVE TT, psum read]
    z2[32u:32u+32] = blk(W2)^T h1   [tile_position column packing]
  p2/t2/h2 like dice1, batched over the whole group.
  scores = h2-as-lhsT @ blkdiag(W3) [2-4 matmuls, tokens-major PSUM]
  wt = sigmoid(scores) * mask
  pool[2-row blocks] = wt^T k       [PE, K=tokens]
  strided copy psum->sbuf, 2 output DMAs (SP queue).
"""
import numpy as np
import ml_dtypes

import concourse.bacc as bacc
import concourse.tile as tile
import concourse.mybir as mybir
import concourse.bass as bass
from concourse.bass_utils import run_bass_kernel_spmd

B, S, D = 2048, 200, 64
H1, H2 = 64, 16
EPS = 1e-9
NCORES = 8
BLOC = B // NCORES          # 256 batch rows per core
NGROUPS = 16

# Per-core slot capacities. The slot with the b-th smallest capacity holds the
# 16 rows of rank band [128b, 128(b+1)) of the globally sorted seq_lengths;
# capacities include a ~+16 margin over the uniform quantile 12.5(b+1)
# (>=9 sigma safety). Ascending order measured fastest under TimelineSim.
CAPS = (32, 48, 64, 80, 80, 96, 112, 128, 144, 144, 160, 176, 192, 192, 208, 208)

F32 = mybir.dt.float32
BF16 = mybir.dt.bfloat16
AF = mybir.ActivationFunctionType
ALU = mybir.AluOpType
bf = ml_dtypes.bfloat16

_CACHE = {}
TRACE = False
LAST_RESULT = None


def _build(caps, lp_bufs=6, kt_bufs=2, qk_bufs=4, wp_bufs=4, h1_bufs=4,
           psx_bufs=4, psp_bufs=1, pss_bufs=1, wt_bufs=4, out_q='sp',
           xbar_inloop=False):
    nc = bacc.Bacc("TRN2", target_bir_lowering=False, debug=False, num_devices=NCORES)
    ngroups = len(caps)
    nb = 16 * ngroups
    npair = nb // 2
    nunit = 4 * ngroups

    key = nc.dram_tensor("key", [nb * S, D], F32, kind="ExternalInput").ap()
    qp = nc.dram_tensor("qp", [128, npair], F32, kind="ExternalInput").ap()
    qwr = nc.dram_tensor("qwr", [2, nunit * 128], BF16, kind="ExternalInput").ap()
    mask = nc.dram_tensor("mask", [128, 32 * ngroups], BF16, kind="ExternalInput").ap()
    wk2 = nc.dram_tensor("wk2", [128, 128], BF16, kind="ExternalInput").ap()
    wqk2 = nc.dram_tensor("wqk2", [128, 128], BF16, kind="ExternalInput").ap()
    w2b = nc.dram_tensor("w2b", [128, 32], BF16, kind="ExternalInput").ap()
    w34 = nc.dram_tensor("w34", [128, 8], BF16, kind="ExternalInput").ap()
    cols = nc.dram_tensor("cols", [128, 8], F32, kind="ExternalInput").ap()
    # cols: 0=s1 1=b1 2=na1 3=a1 4=s2 5=b2 6=na2 7=a2
    dcaps = sorted(set(caps))
    selw = sum(2 * c for c in dcaps)
    selcat = nc.dram_tensor("selcat", [2, selw], BF16, kind="ExternalInput").ap()
    out = nc.dram_tensor("out", [nb, D], F32, kind="ExternalOutput").ap()

    key_r = key.rearrange("(b s) d -> s b d", s=S)  # [200, nb, 64] view

    with tile.TileContext(nc) as tc:
        with (
            tc.tile_pool(name="const", bufs=1) as cp,
            tc.tile_pool(name="load", bufs=lp_bufs) as lp,
            tc.tile_pool(name="kt", bufs=kt_bufs) as ktp,
            tc.tile_pool(name="qk", bufs=qk_bufs) as qkp,
            tc.tile_pool(name="work", bufs=wp_bufs) as wp,
            tc.tile_pool(name="wtpool", bufs=wt_bufs) as wtp,
            tc.tile_pool(name="h1p", bufs=h1_bufs) as h1p,
            tc.tile_pool(name="outp", bufs=2) as op_,
            tc.tile_pool(name="psx", bufs=psx_bufs, space="PSUM") as psx,
            tc.tile_pool(name="psz", bufs=2, space="PSUM") as psz,
            tc.tile_pool(name="pss", bufs=pss_bufs, space="PSUM") as pss,
            tc.tile_pool(name="psp", bufs=psp_bufs, space="PSUM") as psp,
        ):
            # ---- constants into SBUF
            c_qp = cp.tile([128, npair], F32)
            nc.sync.dma_start(out=c_qp[:], in_=qp)
            c_qwr = cp.tile([2, nunit * 128], BF16)
            nc.sync.dma_start(out=c_qwr[:], in_=qwr)
            c_mask = cp.tile([128, 32 * ngroups], BF16)
            nc.sync.dma_start(out=c_mask[:], in_=mask)
            c_wk = cp.tile([128, 128], BF16)
            nc.sync.dma_start(out=c_wk[:], in_=wk2)
            c_wqk = cp.tile([128, 128], BF16)
            nc.sync.dma_start(out=c_wqk[:], in_=wqk2)
            c_w2b = cp.tile([128, 32], BF16)
            nc.sync.dma_start(out=c_w2b[:], in_=w2b)
            c_w34 = cp.tile([128, 8], BF16)
            nc.sync.dma_start(out=c_w34[:], in_=w34)
            c_cols = cp.tile([128, 8], F32)
            nc.sync.dma_start(out=c_cols[:], in_=cols)

            # selector tiles for the qW preload matmul, one per distinct cap:
            # sel[0, 0:c] = 1, sel[1, c:2c] = 1
            c_sel = cp.tile([2, selw], BF16)
            nc.sync.dma_start(out=c_sel[:], in_=selcat)
            sels = {}
            off = 0
            for c in dcaps:
                sels[c] = c_sel[:, off : off + 2 * c]
                off += 2 * c

            unit_t = {}   # per-unit tiles: x1, qk, p1, g1, h1
            grp_t = {}    # per-group tiles: gf, gp, kt, z2, h2, scores, sg, wt, pool, po

            def stage_load(g):
                c = caps[g]
                cf = min(c, 128)
                cpp = c - cf  # partial-chunk rows (0 or 16..80)

                gf = lp.tile([cf, 16, 64], BF16, tag="gf")
                nc.gpsimd.dma_start(out=gf[:], in_=key_r[0:cf, 16 * g : 16 * g + 16, :])
                gp = None
                if cpp:
                    gp = lp.tile([cpp, 16, 64], BF16, tag="gp")
                    prow = min(S - 128, cpp)  # valid HBM token rows beyond 128
                    if prow < cpp:
                        nc.vector.memset(gp[:], 0.0)
                    nc.gpsimd.dma_start(
                        out=gp[0:prow, :, :],
                        in_=bass.AP(
                            key.tensor,
                            (16 * g * S + 128) * D,
                            [[D, prow], [S * D, 16], [1, D]],
                        ),
                    )
                grp_t[g] = {"gf": gf, "gp": gp}

            def st_xbar(g):
                c = caps[g]
                cf = min(c, 128)
                cpp = c - cf
                kt = ktp.tile([128, 8, c], BF16, tag="kt")
                nc.sync.dma_start(
                    out=kt[:, :, 0:cf],
                    in_=grp_t[g]["gf"].rearrange("p b d -> p (b d)"),
                    transpose=True,
                )
                if cpp:
                    nc.sync.dma_start(
                        out=kt[:, :, cf:c],
                        in_=grp_t[g]["gp"].rearrange("p b d -> p (b d)"),
                        transpose=True,
                    )
                grp_t[g]["kt"] = kt

            def st_qk(t):
                g, u = t // 4, t % 4
                c = caps[g]
                kt = grp_t[g]["kt"]
                qk = qkp.tile([128, 2, c], BF16, tag="qk")
                for jl in range(2):
                    jg = 8 * g + 2 * u + jl
                    nc.vector.tensor_scalar(
                        qk[:, jl, :], kt[:, 2 * u + jl, :],
                        c_qp[:, jg : jg + 1], None, ALU.mult,
                    )
                unit_t[t] = {"qk": qk}

            def st_z1(t):
                g, u = t // 4, t % 4
                c = caps[g]
                kt = grp_t[g]["kt"]
                qk = unit_t[t]["qk"]
                ku = 4 * g + u
                # x1 = qW + Wk^T k + Wqk^T (q*k)   (PSUM [128, 2c])
                x1 = psx.tile([128, 2 * c], F32, tag="x1")
                nc.tensor.matmul(x1[:], c_qwr[:, ku * 128 : ku * 128 + 128],
                                 sels[c], start=True, stop=False)
                nc.tensor.matmul(
                    x1[:], c_wk[:],
                    kt[:, 2 * u : 2 * u + 2, :].rearrange("p a b -> p (a b)"),
                    start=False, stop=False)
                nc.tensor.matmul(x1[:], c_wqk[:],
                                 qk.rearrange("p a b -> p (a b)"),
                                 start=False, stop=True)
                unit_t[t]["x1"] = x1

            def st_p1(t):
                g = t // 4
                c = caps[g]
                p1 = wp.tile([128, 2 * c], BF16, tag="p1")
                nc.scalar.activation(p1[:], unit_t[t]["x1"][:], AF.Sigmoid,
                                     bias=c_cols[:, 1:2], scale=c_cols[:, 0:1])
                unit_t[t]["p1"] = p1

            def st_h1(t):
                # dice1: h1 = x1 * (a1 + (1-a1)*p1)
                g = t // 4
                c = caps[g]
                g1 = wp.tile([128, 2 * c], BF16, tag="g1")
                nc.vector.tensor_scalar(g1[:], unit_t[t]["p1"][:], c_cols[:, 2:3],
                                        c_cols[:, 3:4], ALU.mult, ALU.add)
                h1 = h1p.tile([128, 2 * c], BF16, tag="h1")
                nc.vector.tensor_tensor(h1[:], unit_t[t]["x1"][:], g1[:], ALU.mult)
                unit_t[t]["h1"] = h1

            def st_z2(t):
                g, u = t // 4, t % 4
                c = caps[g]
                if u == 0:
                    z2 = psz.tile([128, 2 * c], F32, tag="z2")
                    grp_t[g]["z2"] = z2
                nc.tensor.matmul(grp_t[g]["z2"][32 * u : 32 * u + 32, :], c_w2b[:],
                                 unit_t[t]["h1"][:],
                                 start=True, stop=True, tile_position=(0, 32 * u))
                del unit_t[t]

            def st_p2(g):
                c = caps[g]
                z2 = grp_t[g]["z2"]
                p2 = wp.tile([128, 2 * c], BF16, tag="p2")
                nc.scalar.activation(p2[:], z2[:], AF.Sigmoid,
                                     bias=c_cols[:, 5:6], scale=c_cols[:, 4:5])
                grp_t[g]["p2"] = p2

            def st_h2(g):
                c = caps[g]
                t2 = wp.tile([128, 2 * c], BF16, tag="t2")
                nc.vector.tensor_scalar(t2[:], grp_t[g]["p2"][:], c_cols[:, 6:7],
                                        c_cols[:, 7:8], ALU.mult, ALU.add)
                h2 = wp.tile([128, 2 * c], BF16, tag="h2")
                nc.vector.tensor_tensor(h2[:], grp_t[g]["z2"][:], t2[:], ALU.mult)
                grp_t[g]["h2"] = h2

            def st_scores(g):
                # scores[t, 8jl + 2u + bhat] = sum_h W3[h] h2[32u+16b+h, jl*c+t]
                c = caps[g]
                cf = min(c, 128)
                cpp = c - cf
                h2 = grp_t[g]["h2"]
                ncols = 32 if cpp else 16
                scores = pss.tile([128, ncols], F32, tag="sc")
                if cpp:
                    nc.vector.memset(scores[:], 0.0)
                for jl in range(2):
                    nc.tensor.matmul(scores[0:cf, 8 * jl : 8 * jl + 8],
                                     h2[:, jl * c : jl * c + cf], c_w34[:],
                                     start=True, stop=True)
                    if cpp:
                        nc.tensor.matmul(scores[0:cpp, 16 + 8 * jl : 16 + 8 * jl + 8],
                                         h2[:, jl * c + cf : jl * c + c], c_w34[:],
                                         start=True, stop=True)
                grp_t[g]["scores"] = scores

            def st_sg(g):
                c = caps[g]
                cf = min(c, 128)
                ncols = 32 if c > 128 else 16
                sg = wp.tile([128, ncols], BF16, tag="sg")
                nc.scalar.activation(sg[0:cf, :], grp_t[g]["scores"][0:cf, :],
                                     AF.Sigmoid)
                grp_t[g]["sg"] = sg

            def st_wt(g):
                c = caps[g]
                cf = min(c, 128)
                ncols = 32 if c > 128 else 16
                wt = wtp.tile([128, ncols], BF16, tag="wt")
                nc.vector.tensor_tensor(wt[0:cf, :], grp_t[g]["sg"][0:cf, :],
                                        c_mask[0:cf, 32 * g : 32 * g + ncols],
                                        ALU.mult)
                grp_t[g]["wt"] = wt

            def st_pool(g):
                # pooling: pair p=(2u+jl) covers rows {2p, 2p+1}
                c = caps[g]
                cf = min(c, 128)
                cpp = c - cf
                gf, gp, wt = grp_t[g]["gf"], grp_t[g]["gp"], grp_t[g]["wt"]
                pool = psp.tile([128, 256], F32, tag="pool")
                for p in range(8):
                    u, jl = p // 2, p % 2
                    pb = 32 * u
                    po = 128 * jl
                    rhs_f = gf[:, 2 * p : 2 * p + 2, :].rearrange("p b d -> p (b d)")
                    wcol = 8 * jl + 2 * u
                    nc.tensor.matmul(pool[pb : pb + 2, po : po + 128],
                                     wt[0:cf, wcol : wcol + 2], rhs_f,
                                     start=True, stop=(cpp == 0),
                                     tile_position=(0, pb))
                    if cpp:
                        rhs_p = gp[:, 2 * p : 2 * p + 2, :].rearrange("p b d -> p (b d)")
                        nc.tensor.matmul(pool[pb : pb + 2, po : po + 128],
                                         wt[0:cpp, 16 + wcol : 16 + wcol + 2], rhs_p,
                                         start=False, stop=True,
                                         tile_position=(0, pb))
                grp_t[g]["pool"] = pool

            def st_copy(g):
                # psum rows {32u, 32u+1} -> sbuf (32-aligned copies)
                pool = grp_t[g]["pool"]
                po_sb = op_.tile([128, 256], F32, tag="po")
                nc.scalar.copy(po_sb[0:2, :], pool[0:2, :])
                nc.vector.tensor_copy(po_sb[32:34, :], pool[32:34, :])
                nc.scalar.copy(po_sb[64:66, :], pool[64:66, :])
                nc.vector.tensor_copy(po_sb[96:98, :], pool[96:98, :])
                grp_t[g]["po"] = po_sb

            def st_out(g):
                # out row 16g + 4u + 2jl + bh ; sbuf row 32u+bh, col 128jl+64bh+d
                po_sb = grp_t[g]["po"]
                for bh in range(2):
                    src = po_sb[bh : bh + 97 : 32, :].rearrange(
                        "p (a b) -> p a b", b=128)
                    dst = bass.AP(out.tensor, (16 * g + bh) * D,
                                  [[4 * D, 4], [2 * D, 2], [1, D]])
                    eng = nc.sync if out_q == 'sp' else nc.scalar
                    eng.dma_start(out=dst,
                                  in_=src[:, :, 64 * bh : 64 * bh + 64])
                del grp_t[g]

            # Modulo software pipeline over unit slots t (4 units per group).
            # Engines are strictly in-order; every instruction's producers are
            # emitted >= 1 slot earlier so no engine queue ever blocks.
            T = 4 * ngroups

            def tail_g(t, off):
                # group whose last unit was at slot t-off (tail stage trigger)
                tt = t - off
                return tt // 4 if (0 <= tt < T and tt % 4 == 3) else None

            # front-load every group's key DMA + transpose; the kt/gf ring
            # waits throttle the load/transpose stream to stay just ahead of
            # compute while keeping DMA_ENGINES saturated.
            for g in range(ngroups):
                stage_load(g)
                if not xbar_inloop:
                    st_xbar(g)
            if xbar_inloop:
                st_xbar(0)
                st_xbar(1)
            st_qk(0)
            for t in range(T + 15):
                if xbar_inloop and t % 4 == 0 and t > 0 and t // 4 + 1 < ngroups:
                    st_xbar(t // 4 + 1)
                g = tail_g(t, 12)
                if g is not None:
                    st_out(g)
                g = tail_g(t, 11)
                if g is not None:
                    st_copy(g)
                g = tail_g(t, 10)
                if g is not None:
                    st_pool(g)
                g = tail_g(t, 9)
                if g is not None:
                    st_wt(g)
                g = tail_g(t, 8)
                if g is not None:
                    st_sg(g)
                g = tail_g(t, 7)
                if g is not None:
                    st_scores(g)
                g = tail_g(t, 6)
                if g is not None:
                    st_h2(g)
                g = tail_g(t, 5)
                if g is not None:
                    st_p2(g)
                if 0 <= t - 3 < T:
                    st_z2(t - 3)
                if 0 <= t - 2 < T:
                    st_h1(t - 2)
                if 0 <= t - 1 < T:
                    st_p1(t - 1)
                if t < T:
                    st_z1(t)
                if t + 1 < T:
                    st_qk(t + 1)
    nc.compile()
    return nc


def _prep_consts(W1, alpha1, mean1, var1, W2, alpha2, mean2, var2, W3):
    inv1 = 1.0 / np.sqrt(var1 + EPS)
    inv2 = 1.0 / np.sqrt(var2 + EPS)
    Wq = W1[0:64] + W1[128:192]
    Wk = W1[64:128] - W1[128:192]
    Wqk = W1[192:256]

    def blk(a):
        m = np.zeros((128, 2 * a.shape[1]), np.float32)
        m[0:64, 0 : a.shape[1]] = a
        m[64:128, a.shape[1] :] = a
        return m

    wk2 = blk(Wk).astype(bf)
    wqk2 = blk(Wqk).astype(bf)
    w2b = blk(W2).astype(bf)
    # w34[32u + 16b + h, 2u + b] = W3[h]
    w34 = np.zeros((128, 8), np.float32)
    for u in range(4):
        for b_ in range(2):
            w34[32 * u + 16 * b_ : 32 * u + 16 * b_ + 16, 2 * u + b_] = W3[:, 0]
    w34 = w34.astype(bf)
    cols = np.zeros((128, 8), np.float32)
    cols[:, 0] = np.tile(inv1, 2)
    cols[:, 1] = np.tile(-mean1 * inv1, 2)
    cols[:, 2] = np.tile(1.0 - alpha1, 2)
    cols[:, 3] = np.tile(alpha1, 2)
    cols[:, 4] = np.tile(inv2, 8)
    cols[:, 5] = np.tile(-mean2 * inv2, 8)
    cols[:, 6] = np.tile(1.0 - alpha2, 8)
    cols[:, 7] = np.tile(alpha2, 8)
    return Wq, wk2, wqk2, w2b, w34, cols


def kernel(query_emb, key_emb, seq_length, W1, alpha1, mean1, var1,
           W2, alpha2, mean2, var2, W3, _caps=CAPS):
    (Wq, wk2, wqk2, w2b, w34, cols) = _prep_consts(
        np.asarray(W1, np.float32), np.asarray(alpha1, np.float32),
        np.asarray(mean1, np.float32), np.asarray(var1, np.float32),
        np.asarray(W2, np.float32), np.asarray(alpha2, np.float32),
        np.asarray(mean2, np.float32), np.asarray(var2, np.float32),
        np.asarray(W3, np.float32))
    q = np.asarray(query_emb, np.float32)
    k = np.asarray(key_emb, np.float32)
    sl = np.asarray(seq_length).reshape(-1)
    Btot = q.shape[0]

    ngroups = len(_caps)
    nb = 16 * ngroups
    nunit = 4 * ngroups
    ncores = Btot // nb
    rows_per_slot = 16 * ncores

    if _caps not in _CACHE:
        _CACHE[_caps] = _build(_caps)
    nc = _CACHE[_caps]

    # sort rows ascending by seq_length; the slot with the b-th smallest
    # capacity gets rank band [rows_per_slot*b, rows_per_slot*(b+1)), split 16
    # rows per core.  band_of[kslot] = this slot's rank band index.
    order = np.argsort(sl, kind="stable")
    band_of = np.empty(ngroups, np.int64)
    band_of[np.argsort(np.array(_caps), kind="stable")] = np.arange(ngroups)
    for kslot in range(ngroups):
        b0 = rows_per_slot * int(band_of[kslot])
        band = order[b0 : b0 + rows_per_slot]
        mx = sl[band].max() if band.size else 0
        assert mx <= _caps[kslot], (
            f"slot {kslot} capacity {_caps[kslot]} exceeded by seq_len {mx}")

    qW = q @ Wq  # [B, 64]

    dcaps = sorted(set(_caps))
    selcat = np.zeros((2, sum(2 * c for c in dcaps)), np.float32)
    off = 0
    for c in dcaps:
        selcat[0, off : off + c] = 1.0
        selcat[1, off + c : off + 2 * c] = 1.0
        off += 2 * c
    selcat = selcat.astype(bf)

    def core_rows(cidx):
        return np.concatenate([
            order[rows_per_slot * int(band_of[kslot]) + 16 * cidx :
                  rows_per_slot * int(band_of[kslot]) + 16 * cidx + 16]
            for kslot in range(ngroups)
        ])  # [nb] global row indices, slot-major

    in_maps = []
    for cidx in range(ncores):
        rows = core_rows(cidx)
        qs = q[rows]          # [nb, 64]
        qWs = qW[rows]
        sls = sl[rows]
        # qp[64b + d, pair j] = q[2j + b, d]
        qp_t = np.zeros((128, nb // 2), np.float32)
        for b_ in range(2):
            qp_t[64 * b_ : 64 * b_ + 64] = qs[b_::2].T
        # qwr[jl, 128*(4g+u) + 64b + h] = qW[16g + 4u + 2jl + b, h]
        qwr_t = np.zeros((2, nunit * 128), np.float32)
        qWg = qWs.reshape(ngroups, 4, 2, 2, 64)  # [g, u, jl, b, h]
        qwr_t[0] = qWg[:, :, 0].reshape(nunit, 128).reshape(-1)
        qwr_t[1] = qWg[:, :, 1].reshape(nunit, 128).reshape(-1)
        # mask [128, 32 per group]: col 8jl+2u+b : t < sl (full chunk);
        # col 16+8jl+2u+b : t+128 < sl (partial chunk)
        mk = np.zeros((128, 32 * ngroups), np.float32)
        t_full = np.arange(128)[:, None]
        for gi in range(ngroups):
            slg = sls[16 * gi : 16 * gi + 16]  # local rows 0..16
            cols_full = np.zeros((128, 16), np.float32)
            cols_part = np.zeros((128, 16), np.float32)
            for p in range(8):
                u, jl = p // 2, p % 2
                for b_ in range(2):
                    s_ = slg[2 * p + b_]
                    cc = 8 * jl + 2 * u + b_
                    cols_full[:, cc] = (t_full[:, 0] < s_)
                    cols_part[:, cc] = (t_full[:, 0] + 128 < s_)
            mk[:, 32 * gi : 32 * gi + 16] = cols_full
            mk[:, 32 * gi + 16 : 32 * gi + 32] = cols_part
        in_maps.append({
            "key": k[rows].reshape(nb * S, D),
            "qp": qp_t, "qwr": qwr_t.astype(bf), "mask": mk.astype(bf),
            "wk2": wk2, "wqk2": wqk2, "w2b": w2b, "w34": w34, "cols": cols,
            "selcat": selcat,
        })

    res = run_bass_kernel_spmd(nc, in_maps, list(range(ncores)), trace=TRACE)
    global LAST_RESULT
    LAST_RESULT = res
    full = np.zeros((Btot, D), np.float32)
    for cidx in range(ncores):
        full[core_rows(cidx)] = res.results[cidx]["out"]
    return full
